# revision 29
# baseline (speedup 1.0000x reference)
"""Trainium2 Bass kernel for nn_LogicLayer.

Math: out[b,o] = sum_f softmax(weights[o])[f] * op_f(a,b),
      a = x[b, idx0[o]], b = x[b, idx1[o]].
All 16 logic ops are affine in {1, a, b, ab}, so
      out[b,o] = C0[o] + CA[o]*a + CB[o]*b + CAB[o]*a*b
with per-neuron coefficients Cj[o] = sum_f probs[o,f] * T[f,j].

Strategy (8 NeuronCores, out_dim sharded 8192 neurons/core):
 - Host: transpose x -> x_T [IN_DIM, B] in bf16 so a gathered column of x is
   a contiguous 512B row; split into two 32768-row halves (dma_gather uses
   int16 indices, max 32768 rows).
 - Per core, bucket its 8192 columns by (half(idx0), half(idx1)) so each
   dma_gather call reads one half with int16 indices; pad bucket tails to a
   multiple of 128 with index -1 (trailing negatives are skipped by the
   SWDGE firmware; the padded output slots hold garbage and are dropped on
   the host).
 - Device: SWDGE dma_gather rows of x_T into SBUF [128, slots, 256]
   (position i -> partition i%128, slot i//128). The gather's descriptor
   generation runs on a pair of Q7 cores selected by queue_num; calls are
   round-robined over all 4 SWDGE queues so 4 pairs generate descriptors
   concurrently (4x the single-queue rate, which is the kernel's
   bottleneck). Softmax+coefficient reduction on Scalar/Vector engines,
   then out_col = (C0 + CA*a) + b*(CB + CAB*a) with per-partition
   scale/bias on ScalarE (fp32 temps) and mul/add on VectorE, written back
   as bf16.
 - Host: invert the bucket permutation, upconvert to fp32, transpose back
   to [B, OUT_DIM].
"""

import os

import numpy as np

B = 256
IN_DIM = 65536
OUT_DIM = 65536
NFN = 16
NCORES = 8
SHARD = OUT_DIM // NCORES
HALF = IN_DIM // 2
P = 128

# Coefficient table: op_f(a,b) = T[f,0] + T[f,1]*a + T[f,2]*b + T[f,3]*ab
_T = np.array(
    [
        [0, 0, 0, 0],    # false
        [0, 0, 0, 1],    # a AND b
        [0, 1, 0, -1],   # a AND NOT b
        [0, 1, 0, 0],    # a
        [0, 0, 1, -1],   # NOT a AND b
        [0, 0, 1, 0],    # b
        [0, 1, 1, -2],   # XOR
        [0, 1, 1, -1],   # OR
        [1, -1, -1, 1],  # NOR
        [1, -1, -1, 2],  # XNOR
        [1, 0, -1, 0],   # NOT b
        [1, 0, -1, 1],   # a OR NOT b
        [1, -1, 0, 0],   # NOT a
        [1, -1, 0, 1],   # NOT a OR b
        [1, 0, 0, -1],   # NAND
        [1, 0, 0, 0],    # true
    ],
    dtype=np.float32,
)

_BUILD_CACHE = {}
LAST_RESULTS = None  # BassKernelResults of the most recent run (for profiling)


def _bf16_dtype():
    try:
        import ml_dtypes

        return np.dtype(ml_dtypes.bfloat16)
    except ImportError:
        import jax.numpy as jnp

        return np.dtype(jnp.bfloat16)


def _wrap_idx(idx16):
    """[n] int16 -> [128, n//16] wrapped: position i at (i%16, i//16),
    replicated across the 8 groups of 16 partitions (one per Q7 core)."""
    w = idx16.reshape(-1, 16).T  # [16, n/16]
    return np.ascontiguousarray(np.tile(w, (8, 1)))


def _build_kernel(caps):
    """Build + compile the SPMD program for bucket capacities `caps` (4-tuple,
    each a multiple of 128). Returns (nc, npad)."""
    key = tuple(caps)
    if key in _BUILD_CACHE:
        return _BUILD_CACHE[key]

    import concourse.bacc as bacc
    import concourse.mybir as mybir
    import concourse.tile as tile
    from concourse import library_config

    npad = int(sum(caps))
    nslot = npad // P
    offs = np.concatenate([[0], np.cumsum(caps)]).astype(int)

    nc = bacc.Bacc(
        "TRN2",
        target_bir_lowering=False,
        debug=False,
        dynamic_dma_scratch_size=int(os.environ.get("K_DMA_SCRATCH", "16384")),
        num_swdge_queues=4,
    )
    f32 = mybir.dt.float32
    bf16 = mybir.dt.bfloat16
    i16 = mybir.dt.int16

    xA_d = nc.dram_tensor("xA", [HALF, B], bf16, kind="ExternalInput")
    xB_d = nc.dram_tensor("xB", [HALF, B], bf16, kind="ExternalInput")
    ia_d = nc.dram_tensor("ia", [P, npad // 16], i16, kind="ExternalInput")
    ib_d = nc.dram_tensor("ib", [P, npad // 16], i16, kind="ExternalInput")
    # host pre-wraps w to [P, nslot*NFN] (w[p, s*16+f] = w_orig[s*128+p, f])
    # so the load is one contiguous descriptor per partition instead of an
    # ~npad-descriptor strided storm on the HWDGE queue.
    w_d = nc.dram_tensor("w", [P, (npad // P) * NFN], f32, kind="ExternalInput")
    out_d = nc.dram_tensor("out", [P, nslot * B], bf16, kind="ExternalOutput")

    Exp = mybir.ActivationFunctionType.Exp
    Ident = mybir.ActivationFunctionType.Identity
    X = mybir.AxisListType.X

    # per-chunk gather call ranges: split [c0, c1) at bucket boundaries.
    MAX_CALL = int(os.environ.get("K_MAX_CALL", "1024"))
    NQ = int(os.environ.get("K_NQ", "4"))

    def call_ranges(c0, c1):
        out = []
        for k in range(4):
            lo, hi = max(c0, offs[k]), min(c1, offs[k + 1])
            while lo < hi:
                m = min(hi, lo + MAX_CALL)
                out.append((lo, m, k))
                lo = m
        return out

    from contextlib import ExitStack

    with tile.TileContext(nc) as tc, ExitStack() as ctx:
        nc.gpsimd.load_library(library_config.mlp)
        consts = ctx.enter_context(tc.tile_pool(name="consts", bufs=1))
        work = ctx.enter_context(
            tc.tile_pool(name="work", bufs=int(os.environ.get("K_BUFS", "10")))
        )
        small = ctx.enter_context(tc.tile_pool(name="small", bufs=8))

        # --- load index lists (stay resident) ---
        ia_t = consts.tile([P, npad // 16], i16)
        ib_t = consts.tile([P, npad // 16], i16)
        nc.sync.dma_start(out=ia_t[:], in_=ia_d[:])
        nc.sync.dma_start(out=ib_t[:], in_=ib_d[:])

        # --- softmax -> affine coefficients for all positions ---
        w_t = consts.tile([P, nslot * NFN], f32)
        nc.sync.dma_start(out=w_t[:], in_=w_d[:])
        e_t = consts.tile([P, nslot * NFN], f32)
        nc.scalar.activation(e_t[:], w_t[:], Exp)
        e3 = e_t[:].rearrange("p (s f) -> p s f", f=NFN)

        def rsum(dst, src_ap):
            nc.vector.tensor_reduce(dst, src_ap, axis=X, op=mybir.AluOpType.add)

        s_t = consts.tile([P, nslot], f32)     # sum_f e
        rden = consts.tile([P, nslot], f32)    # 1/sum
        c0_t = consts.tile([P, nslot], f32)
        ca_t = consts.tile([P, nslot], f32)
        cb_t = consts.tile([P, nslot], f32)
        cab_t = consts.tile([P, nslot], f32)
        tmp1 = consts.tile([P, nslot], f32)
        tmp2 = consts.tile([P, nslot], f32)

        rsum(s_t[:], e3)
        nc.vector.reciprocal(out=rden[:], in_=s_t[:])

        # C0: +{8..15}
        rsum(c0_t[:], e3[:, :, 8:16])
        # CA: +{2,3} +{6,7} -{8,9} -{12,13}
        rsum(ca_t[:], e3[:, :, 2:4])
        rsum(tmp1[:], e3[:, :, 6:8])
        nc.vector.tensor_add(ca_t[:], ca_t[:], tmp1[:])
        rsum(tmp1[:], e3[:, :, 8:10])
        nc.vector.tensor_sub(ca_t[:], ca_t[:], tmp1[:])
        rsum(tmp1[:], e3[:, :, 12:14])
        nc.vector.tensor_sub(ca_t[:], ca_t[:], tmp1[:])
        # CB: +{4..7} -{8..11}
        rsum(cb_t[:], e3[:, :, 4:8])
        rsum(tmp1[:], e3[:, :, 8:12])
        nc.vector.tensor_sub(cb_t[:], cb_t[:], tmp1[:])
        # CAB: +e1 -e2 -e4 -2*e6 -e7 +e8 +2*e9 +e11 +e13 -e14
        #    = (e1+e8+e11+e13) - (e2+e4+e7+e14) + 2*(e9-e6)
        def ef(f):
            return e3[:, :, f]

        nc.vector.tensor_add(cab_t[:], ef(1), ef(8))
        nc.vector.tensor_add(cab_t[:], cab_t[:], ef(11))
        nc.vector.tensor_add(cab_t[:], cab_t[:], ef(13))
        nc.vector.tensor_add(tmp1[:], ef(2), ef(4))
        nc.vector.tensor_add(tmp1[:], tmp1[:], ef(7))
        nc.vector.tensor_add(tmp1[:], tmp1[:], ef(14))
        nc.vector.tensor_sub(cab_t[:], cab_t[:], tmp1[:])
        nc.vector.tensor_sub(tmp2[:], ef(9), ef(6))
        nc.vector.tensor_add(cab_t[:], cab_t[:], tmp2[:])
        nc.vector.tensor_add(cab_t[:], cab_t[:], tmp2[:])
        # normalize
        for ct in (c0_t, ca_t, cb_t, cab_t):
            nc.vector.tensor_mul(ct[:], ct[:], rden[:])
        # 32-wide bf16 strips of the t2-path coefficients. The per-chunk t2
        # ops read them with stride-0 only on a middle AP dim and a
        # contiguous 32-elem inner run, which keeps the DVE at full rate (a
        # stride-0 innermost dim halves it). The fp32->bf16 cast runs on
        # ScalarE (cheap there); the broadcast expansion runs on the DVE
        # itself so every later DVE read of the strips is ordered behind it
        # by the engine's in-order stream, independent of the dependency
        # tracker's handling of broadcast APs.
        ca_b = consts.tile([P, nslot], bf16)
        c0_b = consts.tile([P, nslot], bf16)
        nc.scalar.copy(ca_b[:], ca_t[:])
        nc.scalar.copy(c0_b[:], c0_t[:])
        ca32 = consts.tile([P, nslot, 32], bf16)
        c032 = consts.tile([P, nslot, 32], bf16)
        nc.vector.tensor_copy(
            ca32[:], ca_b[:].rearrange("p (s o) -> p s o", o=1)
            .to_broadcast((P, nslot, 32))
        )
        nc.vector.tensor_copy(
            c032[:], c0_b[:].rearrange("p (s o) -> p s o", o=1)
            .to_broadcast((P, nslot, 32))
        )

        # --- main loop over bucket-aligned chunks of columns ---
        chunk_cap = int(os.environ.get("K_CHUNK_POS", "1024"))
        chunks = []
        for k in range(4):
            lo = int(offs[k])
            while lo < offs[k + 1]:
                hi = min(int(offs[k + 1]), lo + chunk_cap)
                chunks.append((lo, hi))
                lo = hi
        # largest chunks first: the kernel tail is the last chunk's
        # gather-drain + compute + writeback, so keep the smallest last
        chunks.sort(key=lambda c: c[0] - c[1])

        qn = 0
        first_chunk = True
        for (p0g, p1g) in chunks:
            cbase, cs = p0g // P, (p1g - p0g) // P
            a_t = work.tile([P, cs, B], bf16)
            b_t = work.tile([P, cs, B], bf16)
            ranges = call_ranges(p0g, p1g)
            if first_chunk:
                # the first dma_gather dispatch holds the Pool sequencer for
                # its whole descriptor generation (cold start); split the
                # leading calls so a 128-index call absorbs that hold and
                # all four Q7 pairs start within ~1us instead of ~9us
                split = []
                for (lo, hi, k) in ranges:
                    if hi - lo > 256:
                        split.append((lo, lo + 128, k))
                        split.append((lo + 128, hi, k))
                    else:
                        split.append((lo, hi, k))
                ranges = split
                first_chunk = False
            for (lo, hi, k) in ranges:
                n = hi - lo
                src = xA_d if k < 2 else xB_d
                srcb = xA_d if k % 2 == 0 else xB_d
                sl = (lo - p0g) // P
                sh = (hi - p0g) // P
                nc.gpsimd.dma_gather(
                    out_ap=a_t[:, sl:sh, :],
                    in_ap=src[:],
                    idxs_ap=ia_t[:, lo // 16 : hi // 16],
                    num_idxs=n,
                    num_idxs_reg=n,
                    elem_size=B,
                    single_packet=True,
                    queue_num=qn % NQ,
                )
                qn += 1
                nc.gpsimd.dma_gather(
                    out_ap=b_t[:, sl:sh, :],
                    in_ap=srcb[:],
                    idxs_ap=ib_t[:, lo // 16 : hi // 16],
                    num_idxs=n,
                    num_idxs_reg=n,
                    elem_size=B,
                    single_packet=True,
                    queue_num=qn % NQ,
                )
                qn += 1
            # the output reuses a_t's buffer: a is fully consumed by the
            # t1/t2 passes before the final add writes it (ordering follows
            # from the t1c/t2c data deps), saving 4KB/partition per buffer
            t1c = work.tile([P, cs, B], bf16)
            t2c = work.tile([P, cs, B], bf16)
            # t1 = CAB*a + CB per slot (ScalarE: per-partition scale/bias)
            for s in range(cs):
                g = cbase + s
                nc.scalar.activation(
                    t1c[:, s, :], a_t[:, s, :], Ident,
                    bias=cb_t[:, g : g + 1], scale=cab_t[:, g : g + 1],
                )
            # t2 = CA*a + C0, split between the engines to balance load:
            # the first T2A slots per-slot on ScalarE, the rest chunk-wide
            # on VectorE (stride-0 broadcast runs at half rate but avoids
            # per-slot overhead and the tensor_scalar PTR-fetch stall).
            sa = min(int(os.environ.get("K_T2ACT", "0")), cs)
            for s in range(sa):
                g = cbase + s
                nc.scalar.activation(
                    t2c[:, s, :], a_t[:, s, :], Ident,
                    bias=c0_t[:, g : g + 1], scale=ca_t[:, g : g + 1],
                )
            if sa < cs:
                gl, gh = cbase + sa, cbase + cs
                nv = cs - sa
                shape4 = (P, nv, B // 32, 32)
                ca_bc = (ca32[:, gl:gh, :]
                         .rearrange("p s (o e) -> p s o e", o=1)
                         .to_broadcast(shape4))
                c0_bc = (c032[:, gl:gh, :]
                         .rearrange("p s (o e) -> p s o e", o=1)
                         .to_broadcast(shape4))
                a4 = a_t[:, sa:cs, :].rearrange("p s (o e) -> p s o e", e=32)
                t4 = t2c[:, sa:cs, :].rearrange("p s (o e) -> p s o e", e=32)
                nc.vector.tensor_mul(t4, a4, ca_bc)
                nc.vector.tensor_add(t4, t4, c0_bc)
            # out = t1*b + t2 chunk-wide (VectorE, full-rate bf16)
            nc.vector.tensor_mul(t1c[:], t1c[:], b_t[:])
            nc.vector.tensor_add(a_t[:], t1c[:], t2c[:])
            nc.sync.dma_start(
                out=out_d[:, cbase * B : (cbase + cs) * B],
                in_=a_t[:].rearrange("p s e -> p (s e)"),
            )

    nc.compile()
    _BUILD_CACHE[key] = (nc, npad)
    return nc, npad


def kernel(x, weights, indices):
    from concourse.bass_utils import run_bass_kernel_spmd

    x = np.asarray(x, dtype=np.float32)
    weights = np.asarray(weights, dtype=np.float32)
    indices = np.asarray(indices, dtype=np.int64)
    bf16 = _bf16_dtype()

    x_T = np.ascontiguousarray(x.T.astype(bf16))  # [IN_DIM, B] bf16
    xA = x_T[:HALF]
    xB = x_T[HALF:]

    # --- per-core bucketing ---
    percore = []
    counts_all = np.zeros((NCORES, 4), dtype=np.int64)
    for c in range(NCORES):
        sl = slice(c * SHARD, (c + 1) * SHARD)
        i0 = indices[0, sl]
        i1 = indices[1, sl]
        bid = (i0 >= HALF).astype(np.int64) * 2 + (i1 >= HALF).astype(np.int64)
        order = np.argsort(bid, kind="stable")
        counts = np.bincount(bid, minlength=4)
        counts_all[c] = counts
        percore.append((sl, i0, i1, bid, order, counts))

    caps = tuple(
        int(-(-int(counts_all[:, k].max()) // P) * P) for k in range(4)
    )
    nc, npad = _build_kernel(caps)
    nslot = npad // P
    offs = np.concatenate([[0], np.cumsum(caps)]).astype(int)

    in_maps = []
    pos_maps = []  # per core: global column index per position (-1 = pad)
    for c in range(NCORES):
        sl, i0, i1, bid, order, counts = percore[c]
        # pad with index 0 (a valid row): trailing -1s would be stripped by
        # the Q7 firmware, but the Pool sequencer's ring bookkeeping still
        # advances by the padded descriptor count, so a stripped call that
        # crosses a 128-index block desyncs the SDMA tail pointer from the
        # ring write offset. Padded output slots are dropped via pos < 0.
        ia = np.zeros(npad, dtype=np.int16)
        ib = np.zeros(npad, dtype=np.int16)
        pos = np.full(npad, -1, dtype=np.int64)
        w_pad = np.zeros((npad, NFN), dtype=np.float32)
        w_shard = weights[sl]
        for k in range(4):
            selk = order[np.searchsorted(bid[order], k) :][: counts[k]]
            o, n = int(offs[k]), int(counts[k])
            ia[o : o + n] = (i0[selk] - (HALF if k >= 2 else 0)).astype(np.int16)
            ib[o : o + n] = (i1[selk] - (HALF if k % 2 else 0)).astype(np.int16)
            pos[o : o + n] = sl.start + selk
            w_pad[o : o + n] = w_shard[selk]
        # wrap w to [P, nslot*NFN] (position s*128+p -> [p, s*16:(s+1)*16])
        # so the device load is contiguous per partition
        w_wrapped = np.ascontiguousarray(
            w_pad.reshape(nslot, P, NFN).transpose(1, 0, 2)
        ).reshape(P, nslot * NFN)
        in_maps.append(
            {
                "xA": xA,
                "xB": xB,
                "ia": _wrap_idx(ia),
                "ib": _wrap_idx(ib),
                "w": w_wrapped,
            }
        )
        pos_maps.append(pos)

    res = run_bass_kernel_spmd(nc, in_maps, core_ids=list(range(NCORES)))
    global LAST_RESULTS
    LAST_RESULTS = res

    out = np.empty((B, OUT_DIM), dtype=np.float32)
    for c in range(NCORES):
        o = np.asarray(res.results[c]["out"]).reshape(P, nslot, B)
        rows = np.ascontiguousarray(o.transpose(1, 0, 2)).reshape(npad, B)
        rows = rows.astype(np.float32)
        pos = pos_maps[c]
        valid = pos >= 0
        out[:, pos[valid]] = rows[valid].T
    return out


# revision 30
# speedup vs baseline: 1.0127x; 1.0127x over previous
"""Trainium2 Bass kernel for nn_LogicLayer.

Math: out[b,o] = sum_f softmax(weights[o])[f] * op_f(a,b),
      a = x[b, idx0[o]], b = x[b, idx1[o]].
All 16 logic ops are affine in {1, a, b, ab}, so
      out[b,o] = C0[o] + CA[o]*a + CB[o]*b + CAB[o]*a*b
with per-neuron coefficients Cj[o] = sum_f probs[o,f] * T[f,j].

Strategy (8 NeuronCores, out_dim sharded 8192 neurons/core):
 - Host: transpose x -> x_T [IN_DIM, B] in bf16 so a gathered column of x is
   a contiguous 512B row; split into two 32768-row halves (dma_gather uses
   int16 indices, max 32768 rows).
 - Per core, bucket its 8192 columns by (half(idx0), half(idx1)) so each
   dma_gather call reads one half with int16 indices; pad bucket tails to a
   multiple of 128 with index -1 (trailing negatives are skipped by the
   SWDGE firmware; the padded output slots hold garbage and are dropped on
   the host).
 - Device: SWDGE dma_gather rows of x_T into SBUF [128, slots, 256]
   (position i -> partition i%128, slot i//128). The gather's descriptor
   generation runs on a pair of Q7 cores selected by queue_num; calls are
   round-robined over all 4 SWDGE queues so 4 pairs generate descriptors
   concurrently (4x the single-queue rate, which is the kernel's
   bottleneck). Softmax+coefficient reduction on Scalar/Vector engines,
   then out_col = (C0 + CA*a) + b*(CB + CAB*a) with per-partition
   scale/bias on ScalarE (fp32 temps) and mul/add on VectorE, written back
   as bf16.
 - Host: invert the bucket permutation, upconvert to fp32, transpose back
   to [B, OUT_DIM].
"""

import os

import numpy as np

B = 256
IN_DIM = 65536
OUT_DIM = 65536
NFN = 16
NCORES = 8
SHARD = OUT_DIM // NCORES
HALF = IN_DIM // 2
P = 128

# Coefficient table: op_f(a,b) = T[f,0] + T[f,1]*a + T[f,2]*b + T[f,3]*ab
_T = np.array(
    [
        [0, 0, 0, 0],    # false
        [0, 0, 0, 1],    # a AND b
        [0, 1, 0, -1],   # a AND NOT b
        [0, 1, 0, 0],    # a
        [0, 0, 1, -1],   # NOT a AND b
        [0, 0, 1, 0],    # b
        [0, 1, 1, -2],   # XOR
        [0, 1, 1, -1],   # OR
        [1, -1, -1, 1],  # NOR
        [1, -1, -1, 2],  # XNOR
        [1, 0, -1, 0],   # NOT b
        [1, 0, -1, 1],   # a OR NOT b
        [1, -1, 0, 0],   # NOT a
        [1, -1, 0, 1],   # NOT a OR b
        [1, 0, 0, -1],   # NAND
        [1, 0, 0, 0],    # true
    ],
    dtype=np.float32,
)

_BUILD_CACHE = {}
LAST_RESULTS = None  # BassKernelResults of the most recent run (for profiling)


def _bf16_dtype():
    try:
        import ml_dtypes

        return np.dtype(ml_dtypes.bfloat16)
    except ImportError:
        import jax.numpy as jnp

        return np.dtype(jnp.bfloat16)


def _wrap_idx(idx16):
    """[n] int16 -> [128, n//16] wrapped: position i at (i%16, i//16),
    replicated across the 8 groups of 16 partitions (one per Q7 core)."""
    w = idx16.reshape(-1, 16).T  # [16, n/16]
    return np.ascontiguousarray(np.tile(w, (8, 1)))


def _build_kernel(caps):
    """Build + compile the SPMD program for bucket capacities `caps` (4-tuple,
    each a multiple of 128). Returns (nc, npad)."""
    key = tuple(caps)
    if key in _BUILD_CACHE:
        return _BUILD_CACHE[key]

    import concourse.bacc as bacc
    import concourse.mybir as mybir
    import concourse.tile as tile
    from concourse import library_config

    npad = int(sum(caps))
    nslot = npad // P
    offs = np.concatenate([[0], np.cumsum(caps)]).astype(int)

    nc = bacc.Bacc(
        "TRN2",
        target_bir_lowering=False,
        debug=False,
        dynamic_dma_scratch_size=int(os.environ.get("K_DMA_SCRATCH", "16384")),
        num_swdge_queues=4,
    )
    f32 = mybir.dt.float32
    bf16 = mybir.dt.bfloat16
    i16 = mybir.dt.int16

    xA_d = nc.dram_tensor("xA", [HALF, B], bf16, kind="ExternalInput")
    xB_d = nc.dram_tensor("xB", [HALF, B], bf16, kind="ExternalInput")
    ia_d = nc.dram_tensor("ia", [P, npad // 16], i16, kind="ExternalInput")
    ib_d = nc.dram_tensor("ib", [P, npad // 16], i16, kind="ExternalInput")
    # host pre-wraps w to [P, nslot*NFN] (w[p, s*16+f] = w_orig[s*128+p, f])
    # so the load is one contiguous descriptor per partition instead of an
    # ~npad-descriptor strided storm on the HWDGE queue.
    w_d = nc.dram_tensor("w", [P, (npad // P) * NFN], f32, kind="ExternalInput")
    out_d = nc.dram_tensor("out", [P, nslot * B], bf16, kind="ExternalOutput")

    Exp = mybir.ActivationFunctionType.Exp
    Ident = mybir.ActivationFunctionType.Identity
    X = mybir.AxisListType.X

    # per-chunk gather call ranges: split [c0, c1) at bucket boundaries.
    MAX_CALL = int(os.environ.get("K_MAX_CALL", "1024"))
    NQ = int(os.environ.get("K_NQ", "4"))

    def call_ranges(c0, c1):
        out = []
        for k in range(4):
            lo, hi = max(c0, offs[k]), min(c1, offs[k + 1])
            while lo < hi:
                m = min(hi, lo + MAX_CALL)
                out.append((lo, m, k))
                lo = m
        return out

    from contextlib import ExitStack

    with tile.TileContext(nc) as tc, ExitStack() as ctx:
        nc.gpsimd.load_library(library_config.mlp)
        consts = ctx.enter_context(tc.tile_pool(name="consts", bufs=1))
        work = ctx.enter_context(
            tc.tile_pool(name="work", bufs=int(os.environ.get("K_BUFS", "10")))
        )
        small = ctx.enter_context(tc.tile_pool(name="small", bufs=8))

        # --- load index lists (stay resident) ---
        ia_t = consts.tile([P, npad // 16], i16)
        ib_t = consts.tile([P, npad // 16], i16)
        nc.sync.dma_start(out=ia_t[:], in_=ia_d[:])
        nc.sync.dma_start(out=ib_t[:], in_=ib_d[:])

        # --- softmax -> affine coefficients for all positions ---
        w_t = consts.tile([P, nslot * NFN], f32)
        nc.sync.dma_start(out=w_t[:], in_=w_d[:])
        e_t = consts.tile([P, nslot * NFN], f32)
        nc.scalar.activation(e_t[:], w_t[:], Exp)
        e3 = e_t[:].rearrange("p (s f) -> p s f", f=NFN)

        def rsum(dst, src_ap):
            nc.vector.tensor_reduce(dst, src_ap, axis=X, op=mybir.AluOpType.add)

        s_t = consts.tile([P, nslot], f32)     # sum_f e
        rden = consts.tile([P, nslot], f32)    # 1/sum
        c0_t = consts.tile([P, nslot], f32)
        ca_t = consts.tile([P, nslot], f32)
        cb_t = consts.tile([P, nslot], f32)
        cab_t = consts.tile([P, nslot], f32)
        tmp1 = consts.tile([P, nslot], f32)
        tmp2 = consts.tile([P, nslot], f32)

        rsum(s_t[:], e3)
        nc.vector.reciprocal(out=rden[:], in_=s_t[:])

        # C0: +{8..15}
        rsum(c0_t[:], e3[:, :, 8:16])
        # CA: +{2,3} +{6,7} -{8,9} -{12,13}
        rsum(ca_t[:], e3[:, :, 2:4])
        rsum(tmp1[:], e3[:, :, 6:8])
        nc.vector.tensor_add(ca_t[:], ca_t[:], tmp1[:])
        rsum(tmp1[:], e3[:, :, 8:10])
        nc.vector.tensor_sub(ca_t[:], ca_t[:], tmp1[:])
        rsum(tmp1[:], e3[:, :, 12:14])
        nc.vector.tensor_sub(ca_t[:], ca_t[:], tmp1[:])
        # CB: +{4..7} -{8..11}
        rsum(cb_t[:], e3[:, :, 4:8])
        rsum(tmp1[:], e3[:, :, 8:12])
        nc.vector.tensor_sub(cb_t[:], cb_t[:], tmp1[:])
        # CAB: +e1 -e2 -e4 -2*e6 -e7 +e8 +2*e9 +e11 +e13 -e14
        #    = (e1+e8+e11+e13) - (e2+e4+e7+e14) + 2*(e9-e6)
        def ef(f):
            return e3[:, :, f]

        nc.vector.tensor_add(cab_t[:], ef(1), ef(8))
        nc.vector.tensor_add(cab_t[:], cab_t[:], ef(11))
        nc.vector.tensor_add(cab_t[:], cab_t[:], ef(13))
        nc.vector.tensor_add(tmp1[:], ef(2), ef(4))
        nc.vector.tensor_add(tmp1[:], tmp1[:], ef(7))
        nc.vector.tensor_add(tmp1[:], tmp1[:], ef(14))
        nc.vector.tensor_sub(cab_t[:], cab_t[:], tmp1[:])
        nc.vector.tensor_sub(tmp2[:], ef(9), ef(6))
        nc.vector.tensor_add(cab_t[:], cab_t[:], tmp2[:])
        nc.vector.tensor_add(cab_t[:], cab_t[:], tmp2[:])
        # normalize
        for ct in (c0_t, ca_t, cb_t, cab_t):
            nc.vector.tensor_mul(ct[:], ct[:], rden[:])
        # 32-wide bf16 strips of the t2-path coefficients. The per-chunk t2
        # ops read them with stride-0 only on a middle AP dim and a
        # contiguous 32-elem inner run, which keeps the DVE at full rate (a
        # stride-0 innermost dim halves it). The fp32->bf16 cast runs on
        # ScalarE (cheap there); the broadcast expansion runs on the DVE
        # itself so every later DVE read of the strips is ordered behind it
        # by the engine's in-order stream, independent of the dependency
        # tracker's handling of broadcast APs.
        ca_b = consts.tile([P, nslot], bf16)
        c0_b = consts.tile([P, nslot], bf16)
        nc.scalar.copy(ca_b[:], ca_t[:])
        nc.scalar.copy(c0_b[:], c0_t[:])
        ca32 = consts.tile([P, nslot, 32], bf16)
        c032 = consts.tile([P, nslot, 32], bf16)
        nc.vector.tensor_copy(
            ca32[:], ca_b[:].rearrange("p (s o) -> p s o", o=1)
            .to_broadcast((P, nslot, 32))
        )
        nc.vector.tensor_copy(
            c032[:], c0_b[:].rearrange("p (s o) -> p s o", o=1)
            .to_broadcast((P, nslot, 32))
        )

        # --- main loop over bucket-aligned chunks of columns ---
        chunk_cap = int(os.environ.get("K_CHUNK_POS", "1024"))
        chunks = []
        for k in range(4):
            lo = int(offs[k])
            while lo < offs[k + 1]:
                hi = min(int(offs[k + 1]), lo + chunk_cap)
                chunks.append((lo, hi))
                lo = hi
        # largest chunks first: the kernel tail is the last chunk's
        # gather-drain + compute + writeback, so keep the smallest last
        chunks.sort(key=lambda c: c[0] - c[1])

        qn = 0
        for (p0g, p1g) in chunks:
            cbase, cs = p0g // P, (p1g - p0g) // P
            a_t = work.tile([P, cs, B], bf16)
            b_t = work.tile([P, cs, B], bf16)
            for (lo, hi, k) in call_ranges(p0g, p1g):
                n = hi - lo
                src = xA_d if k < 2 else xB_d
                srcb = xA_d if k % 2 == 0 else xB_d
                sl = (lo - p0g) // P
                sh = (hi - p0g) // P
                nc.gpsimd.dma_gather(
                    out_ap=a_t[:, sl:sh, :],
                    in_ap=src[:],
                    idxs_ap=ia_t[:, lo // 16 : hi // 16],
                    num_idxs=n,
                    num_idxs_reg=n,
                    elem_size=B,
                    single_packet=True,
                    queue_num=qn % NQ,
                )
                qn += 1
                nc.gpsimd.dma_gather(
                    out_ap=b_t[:, sl:sh, :],
                    in_ap=srcb[:],
                    idxs_ap=ib_t[:, lo // 16 : hi // 16],
                    num_idxs=n,
                    num_idxs_reg=n,
                    elem_size=B,
                    single_packet=True,
                    queue_num=qn % NQ,
                )
                qn += 1
            # the output reuses a_t's buffer: a is fully consumed by the
            # t1/t2 passes before the final add writes it (ordering follows
            # from the t1c/t2c data deps), saving 4KB/partition per buffer
            t1c = work.tile([P, cs, B], bf16)
            t2c = work.tile([P, cs, B], bf16)
            # t1 = CAB*a + CB per slot (ScalarE: per-partition scale/bias)
            for s in range(cs):
                g = cbase + s
                nc.scalar.activation(
                    t1c[:, s, :], a_t[:, s, :], Ident,
                    bias=cb_t[:, g : g + 1], scale=cab_t[:, g : g + 1],
                )
            # t2 = CA*a + C0, split between the engines to balance load:
            # the first T2A slots per-slot on ScalarE, the rest chunk-wide
            # on VectorE (stride-0 broadcast runs at half rate but avoids
            # per-slot overhead and the tensor_scalar PTR-fetch stall).
            sa = min(int(os.environ.get("K_T2ACT", "0")), cs)
            for s in range(sa):
                g = cbase + s
                nc.scalar.activation(
                    t2c[:, s, :], a_t[:, s, :], Ident,
                    bias=c0_t[:, g : g + 1], scale=ca_t[:, g : g + 1],
                )
            if sa < cs:
                gl, gh = cbase + sa, cbase + cs
                nv = cs - sa
                shape4 = (P, nv, B // 32, 32)
                ca_bc = (ca32[:, gl:gh, :]
                         .rearrange("p s (o e) -> p s o e", o=1)
                         .to_broadcast(shape4))
                c0_bc = (c032[:, gl:gh, :]
                         .rearrange("p s (o e) -> p s o e", o=1)
                         .to_broadcast(shape4))
                a4 = a_t[:, sa:cs, :].rearrange("p s (o e) -> p s o e", e=32)
                t4 = t2c[:, sa:cs, :].rearrange("p s (o e) -> p s o e", e=32)
                nc.vector.tensor_mul(t4, a4, ca_bc)
                nc.vector.tensor_add(t4, t4, c0_bc)
            # out = t1*b + t2 chunk-wide (VectorE, full-rate bf16)
            nc.vector.tensor_mul(t1c[:], t1c[:], b_t[:])
            nc.vector.tensor_add(a_t[:], t1c[:], t2c[:])
            nc.sync.dma_start(
                out=out_d[:, cbase * B : (cbase + cs) * B],
                in_=a_t[:].rearrange("p s e -> p (s e)"),
            )

    nc.compile()
    _BUILD_CACHE[key] = (nc, npad)
    return nc, npad


def kernel(x, weights, indices):
    from concourse.bass_utils import run_bass_kernel_spmd

    x = np.asarray(x, dtype=np.float32)
    weights = np.asarray(weights, dtype=np.float32)
    indices = np.asarray(indices, dtype=np.int64)
    bf16 = _bf16_dtype()

    x_T = np.ascontiguousarray(x.T.astype(bf16))  # [IN_DIM, B] bf16
    xA = x_T[:HALF]
    xB = x_T[HALF:]

    # --- per-core bucketing ---
    percore = []
    counts_all = np.zeros((NCORES, 4), dtype=np.int64)
    for c in range(NCORES):
        sl = slice(c * SHARD, (c + 1) * SHARD)
        i0 = indices[0, sl]
        i1 = indices[1, sl]
        bid = (i0 >= HALF).astype(np.int64) * 2 + (i1 >= HALF).astype(np.int64)
        order = np.argsort(bid, kind="stable")
        counts = np.bincount(bid, minlength=4)
        counts_all[c] = counts
        percore.append((sl, i0, i1, bid, order, counts))

    caps = tuple(
        int(-(-int(counts_all[:, k].max()) // P) * P) for k in range(4)
    )
    nc, npad = _build_kernel(caps)
    nslot = npad // P
    offs = np.concatenate([[0], np.cumsum(caps)]).astype(int)

    in_maps = []
    pos_maps = []  # per core: global column index per position (-1 = pad)
    for c in range(NCORES):
        sl, i0, i1, bid, order, counts = percore[c]
        # pad with index 0 (a valid row): trailing -1s would be stripped by
        # the Q7 firmware, but the Pool sequencer's ring bookkeeping still
        # advances by the padded descriptor count, so a stripped call that
        # crosses a 128-index block desyncs the SDMA tail pointer from the
        # ring write offset. Padded output slots are dropped via pos < 0.
        ia = np.zeros(npad, dtype=np.int16)
        ib = np.zeros(npad, dtype=np.int16)
        pos = np.full(npad, -1, dtype=np.int64)
        w_pad = np.zeros((npad, NFN), dtype=np.float32)
        w_shard = weights[sl]
        for k in range(4):
            selk = order[np.searchsorted(bid[order], k) :][: counts[k]]
            o, n = int(offs[k]), int(counts[k])
            ia[o : o + n] = (i0[selk] - (HALF if k >= 2 else 0)).astype(np.int16)
            ib[o : o + n] = (i1[selk] - (HALF if k % 2 else 0)).astype(np.int16)
            pos[o : o + n] = sl.start + selk
            w_pad[o : o + n] = w_shard[selk]
        # wrap w to [P, nslot*NFN] (position s*128+p -> [p, s*16:(s+1)*16])
        # so the device load is contiguous per partition
        w_wrapped = np.ascontiguousarray(
            w_pad.reshape(nslot, P, NFN).transpose(1, 0, 2)
        ).reshape(P, nslot * NFN)
        in_maps.append(
            {
                "xA": xA,
                "xB": xB,
                "ia": _wrap_idx(ia),
                "ib": _wrap_idx(ib),
                "w": w_wrapped,
            }
        )
        pos_maps.append(pos)

    res = run_bass_kernel_spmd(nc, in_maps, core_ids=list(range(NCORES)))
    global LAST_RESULTS
    LAST_RESULTS = res

    out = np.empty((B, OUT_DIM), dtype=np.float32)
    for c in range(NCORES):
        o = np.asarray(res.results[c]["out"]).reshape(P, nslot, B)
        rows = np.ascontiguousarray(o.transpose(1, 0, 2)).reshape(npad, B)
        rows = rows.astype(np.float32)
        pos = pos_maps[c]
        valid = pos >= 0
        out[:, pos[valid]] = rows[valid].T
    return out


# revision 31
# speedup vs baseline: 1.0165x; 1.0038x over previous
"""Trainium2 Bass kernel for nn_LogicLayer.

Math: out[b,o] = sum_f softmax(weights[o])[f] * op_f(a,b),
      a = x[b, idx0[o]], b = x[b, idx1[o]].
All 16 logic ops are affine in {1, a, b, ab}, so
      out[b,o] = C0[o] + CA[o]*a + CB[o]*b + CAB[o]*a*b
with per-neuron coefficients Cj[o] = sum_f probs[o,f] * T[f,j].

Strategy (8 NeuronCores, out_dim sharded 8192 neurons/core):
 - Host: transpose x -> x_T [IN_DIM, B] in bf16 so a gathered column of x is
   a contiguous 512B row; split into two 32768-row halves (dma_gather uses
   int16 indices, max 32768 rows).
 - Per core, bucket its 8192 columns by (half(idx0), half(idx1)) so each
   dma_gather call reads one half with int16 indices; pad bucket tails to a
   multiple of 128 with index -1 (trailing negatives are skipped by the
   SWDGE firmware; the padded output slots hold garbage and are dropped on
   the host).
 - Device: SWDGE dma_gather rows of x_T into SBUF [128, slots, 256]
   (position i -> partition i%128, slot i//128). The gather's descriptor
   generation runs on a pair of Q7 cores selected by queue_num; calls are
   round-robined over all 4 SWDGE queues so 4 pairs generate descriptors
   concurrently (4x the single-queue rate, which is the kernel's
   bottleneck). Softmax+coefficient reduction on Scalar/Vector engines,
   then out_col = (C0 + CA*a) + b*(CB + CAB*a) with per-partition
   scale/bias on ScalarE (fp32 temps) and mul/add on VectorE, written back
   as bf16.
 - Host: invert the bucket permutation, upconvert to fp32, transpose back
   to [B, OUT_DIM].
"""

import os

import numpy as np

B = 256
IN_DIM = 65536
OUT_DIM = 65536
NFN = 16
NCORES = 8
SHARD = OUT_DIM // NCORES
HALF = IN_DIM // 2
P = 128

# Coefficient table: op_f(a,b) = T[f,0] + T[f,1]*a + T[f,2]*b + T[f,3]*ab
_T = np.array(
    [
        [0, 0, 0, 0],    # false
        [0, 0, 0, 1],    # a AND b
        [0, 1, 0, -1],   # a AND NOT b
        [0, 1, 0, 0],    # a
        [0, 0, 1, -1],   # NOT a AND b
        [0, 0, 1, 0],    # b
        [0, 1, 1, -2],   # XOR
        [0, 1, 1, -1],   # OR
        [1, -1, -1, 1],  # NOR
        [1, -1, -1, 2],  # XNOR
        [1, 0, -1, 0],   # NOT b
        [1, 0, -1, 1],   # a OR NOT b
        [1, -1, 0, 0],   # NOT a
        [1, -1, 0, 1],   # NOT a OR b
        [1, 0, 0, -1],   # NAND
        [1, 0, 0, 0],    # true
    ],
    dtype=np.float32,
)

_BUILD_CACHE = {}
LAST_RESULTS = None  # BassKernelResults of the most recent run (for profiling)


def _bf16_dtype():
    try:
        import ml_dtypes

        return np.dtype(ml_dtypes.bfloat16)
    except ImportError:
        import jax.numpy as jnp

        return np.dtype(jnp.bfloat16)


def _wrap_idx(idx16):
    """[n] int16 -> [128, n//16] wrapped: position i at (i%16, i//16),
    replicated across the 8 groups of 16 partitions (one per Q7 core)."""
    w = idx16.reshape(-1, 16).T  # [16, n/16]
    return np.ascontiguousarray(np.tile(w, (8, 1)))


def _build_kernel(caps):
    """Build + compile the SPMD program for bucket capacities `caps` (4-tuple,
    each a multiple of 128). Returns (nc, npad)."""
    key = tuple(caps)
    if key in _BUILD_CACHE:
        return _BUILD_CACHE[key]

    import concourse.bacc as bacc
    import concourse.mybir as mybir
    import concourse.tile as tile
    from concourse import library_config

    npad = int(sum(caps))
    nslot = npad // P
    offs = np.concatenate([[0], np.cumsum(caps)]).astype(int)

    nc = bacc.Bacc(
        "TRN2",
        target_bir_lowering=False,
        debug=False,
        dynamic_dma_scratch_size=int(os.environ.get("K_DMA_SCRATCH", "16384")),
        num_swdge_queues=4,
    )
    f32 = mybir.dt.float32
    bf16 = mybir.dt.bfloat16
    i16 = mybir.dt.int16

    xA_d = nc.dram_tensor("xA", [HALF, B], bf16, kind="ExternalInput")
    xB_d = nc.dram_tensor("xB", [HALF, B], bf16, kind="ExternalInput")
    ia_d = nc.dram_tensor("ia", [P, npad // 16], i16, kind="ExternalInput")
    ib_d = nc.dram_tensor("ib", [P, npad // 16], i16, kind="ExternalInput")
    # host pre-wraps w to [P, nslot*NFN] (w[p, s*16+f] = w_orig[s*128+p, f])
    # so the load is one contiguous descriptor per partition instead of an
    # ~npad-descriptor strided storm on the HWDGE queue.
    w_d = nc.dram_tensor("w", [P, (npad // P) * NFN], f32, kind="ExternalInput")
    out_d = nc.dram_tensor("out", [P, nslot * B], bf16, kind="ExternalOutput")

    Exp = mybir.ActivationFunctionType.Exp
    Ident = mybir.ActivationFunctionType.Identity
    X = mybir.AxisListType.X

    # per-chunk gather call ranges: split [c0, c1) at bucket boundaries.
    MAX_CALL = int(os.environ.get("K_MAX_CALL", "1024"))
    NQ = int(os.environ.get("K_NQ", "4"))

    def call_ranges(c0, c1):
        out = []
        for k in range(4):
            lo, hi = max(c0, offs[k]), min(c1, offs[k + 1])
            while lo < hi:
                m = min(hi, lo + MAX_CALL)
                out.append((lo, m, k))
                lo = m
        return out

    from contextlib import ExitStack

    with tile.TileContext(nc) as tc, ExitStack() as ctx:
        nc.gpsimd.load_library(library_config.mlp)
        consts = ctx.enter_context(tc.tile_pool(name="consts", bufs=1))
        work = ctx.enter_context(
            tc.tile_pool(name="work", bufs=int(os.environ.get("K_BUFS", "10")))
        )
        small = ctx.enter_context(tc.tile_pool(name="small", bufs=8))

        # --- load index lists (stay resident) ---
        ia_t = consts.tile([P, npad // 16], i16)
        ib_t = consts.tile([P, npad // 16], i16)
        nc.sync.dma_start(out=ia_t[:], in_=ia_d[:])
        nc.sync.dma_start(out=ib_t[:], in_=ib_d[:])

        # --- softmax -> affine coefficients for all positions ---
        w_t = consts.tile([P, nslot * NFN], f32)
        nc.sync.dma_start(out=w_t[:], in_=w_d[:])
        e_t = consts.tile([P, nslot * NFN], f32)
        nc.scalar.activation(e_t[:], w_t[:], Exp)
        e3 = e_t[:].rearrange("p (s f) -> p s f", f=NFN)

        def rsum(dst, src_ap):
            nc.vector.tensor_reduce(dst, src_ap, axis=X, op=mybir.AluOpType.add)

        s_t = consts.tile([P, nslot], f32)     # sum_f e
        rden = consts.tile([P, nslot], f32)    # 1/sum
        c0_t = consts.tile([P, nslot], f32)
        ca_t = consts.tile([P, nslot], f32)
        cb_t = consts.tile([P, nslot], f32)
        cab_t = consts.tile([P, nslot], f32)
        tmp1 = consts.tile([P, nslot], f32)
        tmp2 = consts.tile([P, nslot], f32)

        rsum(s_t[:], e3)
        nc.vector.reciprocal(out=rden[:], in_=s_t[:])

        # C0: +{8..15}
        rsum(c0_t[:], e3[:, :, 8:16])
        # CA: +{2,3} +{6,7} -{8,9} -{12,13}
        rsum(ca_t[:], e3[:, :, 2:4])
        rsum(tmp1[:], e3[:, :, 6:8])
        nc.vector.tensor_add(ca_t[:], ca_t[:], tmp1[:])
        rsum(tmp1[:], e3[:, :, 8:10])
        nc.vector.tensor_sub(ca_t[:], ca_t[:], tmp1[:])
        rsum(tmp1[:], e3[:, :, 12:14])
        nc.vector.tensor_sub(ca_t[:], ca_t[:], tmp1[:])
        # CB: +{4..7} -{8..11}
        rsum(cb_t[:], e3[:, :, 4:8])
        rsum(tmp1[:], e3[:, :, 8:12])
        nc.vector.tensor_sub(cb_t[:], cb_t[:], tmp1[:])
        # CAB: +e1 -e2 -e4 -2*e6 -e7 +e8 +2*e9 +e11 +e13 -e14
        #    = (e1+e8+e11+e13) - (e2+e4+e7+e14) + 2*(e9-e6)
        def ef(f):
            return e3[:, :, f]

        nc.vector.tensor_add(cab_t[:], ef(1), ef(8))
        nc.vector.tensor_add(cab_t[:], cab_t[:], ef(11))
        nc.vector.tensor_add(cab_t[:], cab_t[:], ef(13))
        nc.vector.tensor_add(tmp1[:], ef(2), ef(4))
        nc.vector.tensor_add(tmp1[:], tmp1[:], ef(7))
        nc.vector.tensor_add(tmp1[:], tmp1[:], ef(14))
        nc.vector.tensor_sub(cab_t[:], cab_t[:], tmp1[:])
        nc.vector.tensor_sub(tmp2[:], ef(9), ef(6))
        nc.vector.tensor_add(cab_t[:], cab_t[:], tmp2[:])
        nc.vector.tensor_add(cab_t[:], cab_t[:], tmp2[:])
        # normalize
        for ct in (c0_t, ca_t, cb_t, cab_t):
            nc.vector.tensor_mul(ct[:], ct[:], rden[:])
        # 32-wide bf16 strips of the t2-path coefficients. The per-chunk t2
        # ops read them with stride-0 only on a middle AP dim and a
        # contiguous 32-elem inner run, which keeps the DVE at full rate (a
        # stride-0 innermost dim halves it). The fp32->bf16 cast runs on
        # ScalarE (cheap there); the broadcast expansion runs on the DVE
        # itself so every later DVE read of the strips is ordered behind it
        # by the engine's in-order stream, independent of the dependency
        # tracker's handling of broadcast APs.
        ca_b = consts.tile([P, nslot], bf16)
        c0_b = consts.tile([P, nslot], bf16)
        nc.scalar.copy(ca_b[:], ca_t[:])
        nc.scalar.copy(c0_b[:], c0_t[:])
        ca32 = consts.tile([P, nslot, 32], bf16)
        c032 = consts.tile([P, nslot, 32], bf16)
        nc.vector.tensor_copy(
            ca32[:], ca_b[:].rearrange("p (s o) -> p s o", o=1)
            .to_broadcast((P, nslot, 32))
        )
        nc.vector.tensor_copy(
            c032[:], c0_b[:].rearrange("p (s o) -> p s o", o=1)
            .to_broadcast((P, nslot, 32))
        )

        # --- main loop over bucket-aligned chunks of columns ---
        chunk_cap = int(os.environ.get("K_CHUNK_POS", "1024"))
        chunks = []
        for k in range(4):
            lo = int(offs[k])
            while lo < offs[k + 1]:
                hi = min(int(offs[k + 1]), lo + chunk_cap)
                chunks.append((lo, hi))
                lo = hi
        # largest chunks first: the kernel tail is the last chunk's
        # gather-drain + compute + writeback, so keep the smallest last
        chunks.sort(key=lambda c: c[0] - c[1])

        qn = 0
        for (p0g, p1g) in chunks:
            cbase, cs = p0g // P, (p1g - p0g) // P
            a_t = work.tile([P, cs, B], bf16)
            b_t = work.tile([P, cs, B], bf16)
            for (lo, hi, k) in call_ranges(p0g, p1g):
                n = hi - lo
                src = xA_d if k < 2 else xB_d
                srcb = xA_d if k % 2 == 0 else xB_d
                sl = (lo - p0g) // P
                sh = (hi - p0g) // P
                nc.gpsimd.dma_gather(
                    out_ap=a_t[:, sl:sh, :],
                    in_ap=src[:],
                    idxs_ap=ia_t[:, lo // 16 : hi // 16],
                    num_idxs=n,
                    num_idxs_reg=n,
                    elem_size=B,
                    single_packet=os.environ.get("K_SP", "0") == "1",
                    queue_num=qn % NQ,
                )
                qn += 1
                nc.gpsimd.dma_gather(
                    out_ap=b_t[:, sl:sh, :],
                    in_ap=srcb[:],
                    idxs_ap=ib_t[:, lo // 16 : hi // 16],
                    num_idxs=n,
                    num_idxs_reg=n,
                    elem_size=B,
                    single_packet=os.environ.get("K_SP", "0") == "1",
                    queue_num=qn % NQ,
                )
                qn += 1
            # the output reuses a_t's buffer: a is fully consumed by the
            # t1/t2 passes before the final add writes it (ordering follows
            # from the t1c/t2c data deps), saving 4KB/partition per buffer
            t1c = work.tile([P, cs, B], bf16)
            t2c = work.tile([P, cs, B], bf16)
            # t1 = CAB*a + CB per slot (ScalarE: per-partition scale/bias)
            for s in range(cs):
                g = cbase + s
                nc.scalar.activation(
                    t1c[:, s, :], a_t[:, s, :], Ident,
                    bias=cb_t[:, g : g + 1], scale=cab_t[:, g : g + 1],
                )
            # t2 = CA*a + C0, split between the engines to balance load:
            # the first T2A slots per-slot on ScalarE, the rest chunk-wide
            # on VectorE (stride-0 broadcast runs at half rate but avoids
            # per-slot overhead and the tensor_scalar PTR-fetch stall).
            sa = min(int(os.environ.get("K_T2ACT", "0")), cs)
            for s in range(sa):
                g = cbase + s
                nc.scalar.activation(
                    t2c[:, s, :], a_t[:, s, :], Ident,
                    bias=c0_t[:, g : g + 1], scale=ca_t[:, g : g + 1],
                )
            if sa < cs:
                gl, gh = cbase + sa, cbase + cs
                nv = cs - sa
                shape4 = (P, nv, B // 32, 32)
                ca_bc = (ca32[:, gl:gh, :]
                         .rearrange("p s (o e) -> p s o e", o=1)
                         .to_broadcast(shape4))
                c0_bc = (c032[:, gl:gh, :]
                         .rearrange("p s (o e) -> p s o e", o=1)
                         .to_broadcast(shape4))
                a4 = a_t[:, sa:cs, :].rearrange("p s (o e) -> p s o e", e=32)
                t4 = t2c[:, sa:cs, :].rearrange("p s (o e) -> p s o e", e=32)
                nc.vector.tensor_mul(t4, a4, ca_bc)
                nc.vector.tensor_add(t4, t4, c0_bc)
            # out = t1*b + t2 chunk-wide (VectorE, full-rate bf16)
            nc.vector.tensor_mul(t1c[:], t1c[:], b_t[:])
            nc.vector.tensor_add(a_t[:], t1c[:], t2c[:])
            nc.sync.dma_start(
                out=out_d[:, cbase * B : (cbase + cs) * B],
                in_=a_t[:].rearrange("p s e -> p (s e)"),
            )

    nc.compile()
    _BUILD_CACHE[key] = (nc, npad)
    return nc, npad


def kernel(x, weights, indices):
    from concourse.bass_utils import run_bass_kernel_spmd

    x = np.asarray(x, dtype=np.float32)
    weights = np.asarray(weights, dtype=np.float32)
    indices = np.asarray(indices, dtype=np.int64)
    bf16 = _bf16_dtype()

    x_T = np.ascontiguousarray(x.T.astype(bf16))  # [IN_DIM, B] bf16
    xA = x_T[:HALF]
    xB = x_T[HALF:]

    # --- per-core bucketing ---
    percore = []
    counts_all = np.zeros((NCORES, 4), dtype=np.int64)
    for c in range(NCORES):
        sl = slice(c * SHARD, (c + 1) * SHARD)
        i0 = indices[0, sl]
        i1 = indices[1, sl]
        bid = (i0 >= HALF).astype(np.int64) * 2 + (i1 >= HALF).astype(np.int64)
        order = np.argsort(bid, kind="stable")
        counts = np.bincount(bid, minlength=4)
        counts_all[c] = counts
        percore.append((sl, i0, i1, bid, order, counts))

    caps = tuple(
        int(-(-int(counts_all[:, k].max()) // P) * P) for k in range(4)
    )
    nc, npad = _build_kernel(caps)
    nslot = npad // P
    offs = np.concatenate([[0], np.cumsum(caps)]).astype(int)

    in_maps = []
    pos_maps = []  # per core: global column index per position (-1 = pad)
    for c in range(NCORES):
        sl, i0, i1, bid, order, counts = percore[c]
        # pad with index 0 (a valid row): trailing -1s would be stripped by
        # the Q7 firmware, but the Pool sequencer's ring bookkeeping still
        # advances by the padded descriptor count, so a stripped call that
        # crosses a 128-index block desyncs the SDMA tail pointer from the
        # ring write offset. Padded output slots are dropped via pos < 0.
        ia = np.zeros(npad, dtype=np.int16)
        ib = np.zeros(npad, dtype=np.int16)
        pos = np.full(npad, -1, dtype=np.int64)
        w_pad = np.zeros((npad, NFN), dtype=np.float32)
        w_shard = weights[sl]
        for k in range(4):
            selk = order[np.searchsorted(bid[order], k) :][: counts[k]]
            o, n = int(offs[k]), int(counts[k])
            ia[o : o + n] = (i0[selk] - (HALF if k >= 2 else 0)).astype(np.int16)
            ib[o : o + n] = (i1[selk] - (HALF if k % 2 else 0)).astype(np.int16)
            pos[o : o + n] = sl.start + selk
            w_pad[o : o + n] = w_shard[selk]
        # wrap w to [P, nslot*NFN] (position s*128+p -> [p, s*16:(s+1)*16])
        # so the device load is contiguous per partition
        w_wrapped = np.ascontiguousarray(
            w_pad.reshape(nslot, P, NFN).transpose(1, 0, 2)
        ).reshape(P, nslot * NFN)
        in_maps.append(
            {
                "xA": xA,
                "xB": xB,
                "ia": _wrap_idx(ia),
                "ib": _wrap_idx(ib),
                "w": w_wrapped,
            }
        )
        pos_maps.append(pos)

    res = run_bass_kernel_spmd(nc, in_maps, core_ids=list(range(NCORES)))
    global LAST_RESULTS
    LAST_RESULTS = res

    out = np.empty((B, OUT_DIM), dtype=np.float32)
    for c in range(NCORES):
        o = np.asarray(res.results[c]["out"]).reshape(P, nslot, B)
        rows = np.ascontiguousarray(o.transpose(1, 0, 2)).reshape(npad, B)
        rows = rows.astype(np.float32)
        pos = pos_maps[c]
        valid = pos >= 0
        out[:, pos[valid]] = rows[valid].T
    return out


# revision 32
# speedup vs baseline: 1.1582x; 1.1394x over previous
"""Trainium2 Bass kernel for nn_LogicLayer.

Math: out[b,o] = sum_f softmax(weights[o])[f] * op_f(a,b),
      a = x[b, idx0[o]], b = x[b, idx1[o]].
All 16 logic ops are affine in {1, a, b, ab}, so
      out[b,o] = C0[o] + CA[o]*a + CB[o]*b + CAB[o]*a*b
with per-neuron coefficients Cj[o] = sum_f probs[o,f] * T[f,j].

Strategy (8 NeuronCores, out_dim sharded 8192 neurons/core):
 - Host: transpose x -> x_T [IN_DIM, B] in bf16 so a gathered column of x is
   a contiguous 512B row; split into two 32768-row halves (dma_gather uses
   int16 indices, max 32768 rows).
 - Per core, bucket its 8192 columns by (half(idx0), half(idx1)) so each
   dma_gather call reads one half with int16 indices; pad bucket tails to a
   multiple of 128 with index -1 (trailing negatives are skipped by the
   SWDGE firmware; the padded output slots hold garbage and are dropped on
   the host).
 - Device: SWDGE dma_gather rows of x_T into SBUF [128, slots, 256]
   (position i -> partition i%128, slot i//128). The gather's descriptor
   generation runs on a pair of Q7 cores selected by queue_num; calls are
   round-robined over all 4 SWDGE queues so 4 pairs generate descriptors
   concurrently (4x the single-queue rate, which is the kernel's
   bottleneck). Softmax+coefficient reduction on Scalar/Vector engines,
   then out_col = (C0 + CA*a) + b*(CB + CAB*a) with per-partition
   scale/bias on ScalarE (fp32 temps) and mul/add on VectorE, written back
   as bf16.
 - Host: invert the bucket permutation, upconvert to fp32, transpose back
   to [B, OUT_DIM].
"""

import os

import numpy as np

B = 256
IN_DIM = 65536
OUT_DIM = 65536
NFN = 16
NCORES = 8
SHARD = OUT_DIM // NCORES
HALF = IN_DIM // 2
P = 128

# Coefficient table: op_f(a,b) = T[f,0] + T[f,1]*a + T[f,2]*b + T[f,3]*ab
_T = np.array(
    [
        [0, 0, 0, 0],    # false
        [0, 0, 0, 1],    # a AND b
        [0, 1, 0, -1],   # a AND NOT b
        [0, 1, 0, 0],    # a
        [0, 0, 1, -1],   # NOT a AND b
        [0, 0, 1, 0],    # b
        [0, 1, 1, -2],   # XOR
        [0, 1, 1, -1],   # OR
        [1, -1, -1, 1],  # NOR
        [1, -1, -1, 2],  # XNOR
        [1, 0, -1, 0],   # NOT b
        [1, 0, -1, 1],   # a OR NOT b
        [1, -1, 0, 0],   # NOT a
        [1, -1, 0, 1],   # NOT a OR b
        [1, 0, 0, -1],   # NAND
        [1, 0, 0, 0],    # true
    ],
    dtype=np.float32,
)

_BUILD_CACHE = {}
LAST_RESULTS = None  # BassKernelResults of the most recent run (for profiling)


def _bf16_dtype():
    try:
        import ml_dtypes

        return np.dtype(ml_dtypes.bfloat16)
    except ImportError:
        import jax.numpy as jnp

        return np.dtype(jnp.bfloat16)


def _wrap_idx(idx16):
    """[n] int16 -> [128, n//16] wrapped: position i at (i%16, i//16),
    replicated across the 8 groups of 16 partitions (one per Q7 core)."""
    w = idx16.reshape(-1, 16).T  # [16, n/16]
    return np.ascontiguousarray(np.tile(w, (8, 1)))


def _build_kernel(caps):
    """Build + compile the SPMD program for bucket capacities `caps` (4-tuple,
    each a multiple of 128). Returns (nc, npad)."""
    key = tuple(caps)
    if key in _BUILD_CACHE:
        return _BUILD_CACHE[key]

    import concourse.bacc as bacc
    import concourse.mybir as mybir
    import concourse.tile as tile
    from concourse import library_config

    npad = int(sum(caps))
    nslot = npad // P
    offs = np.concatenate([[0], np.cumsum(caps)]).astype(int)

    nc = bacc.Bacc(
        "TRN2",
        target_bir_lowering=False,
        debug=False,
        dynamic_dma_scratch_size=int(os.environ.get("K_DMA_SCRATCH", "16384")),
        num_swdge_queues=4,
    )
    f32 = mybir.dt.float32
    bf16 = mybir.dt.bfloat16
    i16 = mybir.dt.int16

    xA_d = nc.dram_tensor("xA", [HALF, B], bf16, kind="ExternalInput")
    xB_d = nc.dram_tensor("xB", [HALF, B], bf16, kind="ExternalInput")
    ia_d = nc.dram_tensor("ia", [P, npad // 16], i16, kind="ExternalInput")
    ib_d = nc.dram_tensor("ib", [P, npad // 16], i16, kind="ExternalInput")
    # host pre-wraps w to [P, nslot*NFN] (w[p, s*16+f] = w_orig[s*128+p, f])
    # so the load is one contiguous descriptor per partition instead of an
    # ~npad-descriptor strided storm on the HWDGE queue.
    w_d = nc.dram_tensor("w", [P, (npad // P) * NFN], f32, kind="ExternalInput")
    out_d = nc.dram_tensor("out", [P, nslot * B], bf16, kind="ExternalOutput")

    Exp = mybir.ActivationFunctionType.Exp
    Ident = mybir.ActivationFunctionType.Identity
    X = mybir.AxisListType.X

    # per-chunk gather call ranges: split [c0, c1) at bucket boundaries.
    MAX_CALL = int(os.environ.get("K_MAX_CALL", "1024"))
    NQ = int(os.environ.get("K_NQ", "4"))

    def call_ranges(c0, c1):
        out = []
        for k in range(4):
            lo, hi = max(c0, offs[k]), min(c1, offs[k + 1])
            while lo < hi:
                m = min(hi, lo + MAX_CALL)
                out.append((lo, m, k))
                lo = m
        return out

    from contextlib import ExitStack

    with tile.TileContext(nc) as tc, ExitStack() as ctx:
        nc.gpsimd.load_library(library_config.mlp)
        consts = ctx.enter_context(tc.tile_pool(name="consts", bufs=1))
        work = ctx.enter_context(
            tc.tile_pool(name="work", bufs=int(os.environ.get("K_BUFS", "10")))
        )
        small = ctx.enter_context(tc.tile_pool(name="small", bufs=8))

        # --- load index lists (stay resident) ---
        ia_t = consts.tile([P, npad // 16], i16)
        ib_t = consts.tile([P, npad // 16], i16)
        nc.sync.dma_start(out=ia_t[:], in_=ia_d[:])
        nc.sync.dma_start(out=ib_t[:], in_=ib_d[:])

        # --- softmax -> affine coefficients for all positions ---
        w_t = consts.tile([P, nslot * NFN], f32)
        nc.sync.dma_start(out=w_t[:], in_=w_d[:])
        e_t = consts.tile([P, nslot * NFN], f32)
        nc.scalar.activation(e_t[:], w_t[:], Exp)
        e3 = e_t[:].rearrange("p (s f) -> p s f", f=NFN)

        def rsum(dst, src_ap):
            nc.vector.tensor_reduce(dst, src_ap, axis=X, op=mybir.AluOpType.add)

        s_t = consts.tile([P, nslot], f32)     # sum_f e
        rden = consts.tile([P, nslot], f32)    # 1/sum
        c0_t = consts.tile([P, nslot], f32)
        ca_t = consts.tile([P, nslot], f32)
        cb_t = consts.tile([P, nslot], f32)
        cab_t = consts.tile([P, nslot], f32)
        tmp1 = consts.tile([P, nslot], f32)
        tmp2 = consts.tile([P, nslot], f32)

        rsum(s_t[:], e3)
        nc.vector.reciprocal(out=rden[:], in_=s_t[:])

        # C0: +{8..15}
        rsum(c0_t[:], e3[:, :, 8:16])
        # CA: +{2,3} +{6,7} -{8,9} -{12,13}
        rsum(ca_t[:], e3[:, :, 2:4])
        rsum(tmp1[:], e3[:, :, 6:8])
        nc.vector.tensor_add(ca_t[:], ca_t[:], tmp1[:])
        rsum(tmp1[:], e3[:, :, 8:10])
        nc.vector.tensor_sub(ca_t[:], ca_t[:], tmp1[:])
        rsum(tmp1[:], e3[:, :, 12:14])
        nc.vector.tensor_sub(ca_t[:], ca_t[:], tmp1[:])
        # CB: +{4..7} -{8..11}
        rsum(cb_t[:], e3[:, :, 4:8])
        rsum(tmp1[:], e3[:, :, 8:12])
        nc.vector.tensor_sub(cb_t[:], cb_t[:], tmp1[:])
        # CAB: +e1 -e2 -e4 -2*e6 -e7 +e8 +2*e9 +e11 +e13 -e14
        #    = (e1+e8+e11+e13) - (e2+e4+e7+e14) + 2*(e9-e6)
        def ef(f):
            return e3[:, :, f]

        nc.vector.tensor_add(cab_t[:], ef(1), ef(8))
        nc.vector.tensor_add(cab_t[:], cab_t[:], ef(11))
        nc.vector.tensor_add(cab_t[:], cab_t[:], ef(13))
        nc.vector.tensor_add(tmp1[:], ef(2), ef(4))
        nc.vector.tensor_add(tmp1[:], tmp1[:], ef(7))
        nc.vector.tensor_add(tmp1[:], tmp1[:], ef(14))
        nc.vector.tensor_sub(cab_t[:], cab_t[:], tmp1[:])
        nc.vector.tensor_sub(tmp2[:], ef(9), ef(6))
        nc.vector.tensor_add(cab_t[:], cab_t[:], tmp2[:])
        nc.vector.tensor_add(cab_t[:], cab_t[:], tmp2[:])
        # normalize
        for ct in (c0_t, ca_t, cb_t, cab_t):
            nc.vector.tensor_mul(ct[:], ct[:], rden[:])
        # 32-wide bf16 strips of the t2-path coefficients. The per-chunk t2
        # ops read them with stride-0 only on a middle AP dim and a
        # contiguous 32-elem inner run, which keeps the DVE at full rate (a
        # stride-0 innermost dim halves it). The fp32->bf16 cast runs on
        # ScalarE (cheap there); the broadcast expansion runs on the DVE
        # itself so every later DVE read of the strips is ordered behind it
        # by the engine's in-order stream, independent of the dependency
        # tracker's handling of broadcast APs.
        ca_b = consts.tile([P, nslot], bf16)
        c0_b = consts.tile([P, nslot], bf16)
        nc.scalar.copy(ca_b[:], ca_t[:])
        nc.scalar.copy(c0_b[:], c0_t[:])
        ca32 = consts.tile([P, nslot, 32], bf16)
        c032 = consts.tile([P, nslot, 32], bf16)
        nc.vector.tensor_copy(
            ca32[:], ca_b[:].rearrange("p (s o) -> p s o", o=1)
            .to_broadcast((P, nslot, 32))
        )
        nc.vector.tensor_copy(
            c032[:], c0_b[:].rearrange("p (s o) -> p s o", o=1)
            .to_broadcast((P, nslot, 32))
        )

        # --- main loop over bucket-aligned chunks of columns ---
        chunk_cap = int(os.environ.get("K_CHUNK_POS", "1024"))
        chunks = []
        for k in range(4):
            lo = int(offs[k])
            while lo < offs[k + 1]:
                hi = min(int(offs[k + 1]), lo + chunk_cap)
                chunks.append((lo, hi))
                lo = hi
        # largest chunks first: the kernel tail is the last chunk's
        # gather-drain + compute + writeback, so keep the smallest last
        chunks.sort(key=lambda c: c[0] - c[1])

        qn = 0
        for (p0g, p1g) in chunks:
            cbase, cs = p0g // P, (p1g - p0g) // P
            a_t = work.tile([P, cs, B], bf16)
            b_t = work.tile([P, cs, B], bf16)
            for (lo, hi, k) in call_ranges(p0g, p1g):
                n = hi - lo
                src = xA_d if k < 2 else xB_d
                srcb = xA_d if k % 2 == 0 else xB_d
                sl = (lo - p0g) // P
                sh = (hi - p0g) // P
                nc.gpsimd.dma_gather(
                    out_ap=a_t[:, sl:sh, :],
                    in_ap=src[:],
                    idxs_ap=ia_t[:, lo // 16 : hi // 16],
                    num_idxs=n,
                    num_idxs_reg=n,
                    elem_size=B,
                    single_packet=os.environ.get("K_SP", "1") == "1",
                    queue_num=qn % NQ,
                )
                qn += 1
                nc.gpsimd.dma_gather(
                    out_ap=b_t[:, sl:sh, :],
                    in_ap=srcb[:],
                    idxs_ap=ib_t[:, lo // 16 : hi // 16],
                    num_idxs=n,
                    num_idxs_reg=n,
                    elem_size=B,
                    single_packet=os.environ.get("K_SP", "1") == "1",
                    queue_num=qn % NQ,
                )
                qn += 1
            # the output reuses a_t's buffer: a is fully consumed by the
            # t1/t2 passes before the final add writes it (ordering follows
            # from the t1c/t2c data deps), saving 4KB/partition per buffer
            t1c = work.tile([P, cs, B], bf16)
            t2c = work.tile([P, cs, B], bf16)
            # t1 = CAB*a + CB per slot (ScalarE: per-partition scale/bias)
            for s in range(cs):
                g = cbase + s
                nc.scalar.activation(
                    t1c[:, s, :], a_t[:, s, :], Ident,
                    bias=cb_t[:, g : g + 1], scale=cab_t[:, g : g + 1],
                )
            # t2 = CA*a + C0, split between the engines to balance load:
            # the first T2A slots per-slot on ScalarE, the rest chunk-wide
            # on VectorE (stride-0 broadcast runs at half rate but avoids
            # per-slot overhead and the tensor_scalar PTR-fetch stall).
            sa = min(int(os.environ.get("K_T2ACT", "0")), cs)
            for s in range(sa):
                g = cbase + s
                nc.scalar.activation(
                    t2c[:, s, :], a_t[:, s, :], Ident,
                    bias=c0_t[:, g : g + 1], scale=ca_t[:, g : g + 1],
                )
            if sa < cs:
                gl, gh = cbase + sa, cbase + cs
                nv = cs - sa
                shape4 = (P, nv, B // 32, 32)
                ca_bc = (ca32[:, gl:gh, :]
                         .rearrange("p s (o e) -> p s o e", o=1)
                         .to_broadcast(shape4))
                c0_bc = (c032[:, gl:gh, :]
                         .rearrange("p s (o e) -> p s o e", o=1)
                         .to_broadcast(shape4))
                a4 = a_t[:, sa:cs, :].rearrange("p s (o e) -> p s o e", e=32)
                t4 = t2c[:, sa:cs, :].rearrange("p s (o e) -> p s o e", e=32)
                nc.vector.tensor_mul(t4, a4, ca_bc)
                nc.vector.tensor_add(t4, t4, c0_bc)
            # out = t1*b + t2 chunk-wide (VectorE, full-rate bf16)
            nc.vector.tensor_mul(t1c[:], t1c[:], b_t[:])
            nc.vector.tensor_add(a_t[:], t1c[:], t2c[:])
            nc.sync.dma_start(
                out=out_d[:, cbase * B : (cbase + cs) * B],
                in_=a_t[:].rearrange("p s e -> p (s e)"),
            )

    nc.compile()
    _BUILD_CACHE[key] = (nc, npad)
    return nc, npad


def kernel(x, weights, indices):
    from concourse.bass_utils import run_bass_kernel_spmd

    x = np.asarray(x, dtype=np.float32)
    weights = np.asarray(weights, dtype=np.float32)
    indices = np.asarray(indices, dtype=np.int64)
    bf16 = _bf16_dtype()

    x_T = np.ascontiguousarray(x.T.astype(bf16))  # [IN_DIM, B] bf16
    xA = x_T[:HALF]
    xB = x_T[HALF:]

    # --- per-core bucketing ---
    percore = []
    counts_all = np.zeros((NCORES, 4), dtype=np.int64)
    for c in range(NCORES):
        sl = slice(c * SHARD, (c + 1) * SHARD)
        i0 = indices[0, sl]
        i1 = indices[1, sl]
        bid = (i0 >= HALF).astype(np.int64) * 2 + (i1 >= HALF).astype(np.int64)
        order = np.argsort(bid, kind="stable")
        counts = np.bincount(bid, minlength=4)
        counts_all[c] = counts
        percore.append((sl, i0, i1, bid, order, counts))

    caps = tuple(
        int(-(-int(counts_all[:, k].max()) // P) * P) for k in range(4)
    )
    nc, npad = _build_kernel(caps)
    nslot = npad // P
    offs = np.concatenate([[0], np.cumsum(caps)]).astype(int)

    in_maps = []
    pos_maps = []  # per core: global column index per position (-1 = pad)
    for c in range(NCORES):
        sl, i0, i1, bid, order, counts = percore[c]
        # pad with index 0 (a valid row): trailing -1s would be stripped by
        # the Q7 firmware, but the Pool sequencer's ring bookkeeping still
        # advances by the padded descriptor count, so a stripped call that
        # crosses a 128-index block desyncs the SDMA tail pointer from the
        # ring write offset. Padded output slots are dropped via pos < 0.
        ia = np.zeros(npad, dtype=np.int16)
        ib = np.zeros(npad, dtype=np.int16)
        pos = np.full(npad, -1, dtype=np.int64)
        w_pad = np.zeros((npad, NFN), dtype=np.float32)
        w_shard = weights[sl]
        for k in range(4):
            selk = order[np.searchsorted(bid[order], k) :][: counts[k]]
            o, n = int(offs[k]), int(counts[k])
            ia[o : o + n] = (i0[selk] - (HALF if k >= 2 else 0)).astype(np.int16)
            ib[o : o + n] = (i1[selk] - (HALF if k % 2 else 0)).astype(np.int16)
            pos[o : o + n] = sl.start + selk
            w_pad[o : o + n] = w_shard[selk]
        # wrap w to [P, nslot*NFN] (position s*128+p -> [p, s*16:(s+1)*16])
        # so the device load is contiguous per partition
        w_wrapped = np.ascontiguousarray(
            w_pad.reshape(nslot, P, NFN).transpose(1, 0, 2)
        ).reshape(P, nslot * NFN)
        in_maps.append(
            {
                "xA": xA,
                "xB": xB,
                "ia": _wrap_idx(ia),
                "ib": _wrap_idx(ib),
                "w": w_wrapped,
            }
        )
        pos_maps.append(pos)

    res = run_bass_kernel_spmd(nc, in_maps, core_ids=list(range(NCORES)))
    global LAST_RESULTS
    LAST_RESULTS = res

    out = np.empty((B, OUT_DIM), dtype=np.float32)
    for c in range(NCORES):
        o = np.asarray(res.results[c]["out"]).reshape(P, nslot, B)
        rows = np.ascontiguousarray(o.transpose(1, 0, 2)).reshape(npad, B)
        rows = rows.astype(np.float32)
        pos = pos_maps[c]
        valid = pos >= 0
        out[:, pos[valid]] = rows[valid].T
    return out
